# revision 10
# baseline (speedup 1.0000x reference)
"""NT-Xent (SimCLR contrastive) loss on Trainium2, sharded across 8 NeuronCores.

Sharding: each core computes a [512, 4096] row-slice of the similarity matrix.
Host ships z^T (bf16, transposed layout only — no host arithmetic) shared to
all cores plus per-core own/partner column slices; per-core scalar partials
are summed on the host (the unshard step).

Device pipeline (per core, SPMD):
  - column norms: squares (GpSimd) + all-ones matmul = partition-reduce with
    free broadcast; rinv16 = exp(-0.5*ln(ssq) + ln 16)  [one ACT table set]
  - zn16 = z * rinv16 -> fp8e4 (x16 scaling keeps values out of fp8 denormals)
  - main Gram slice: fp8 DoubleRow matmuls (2 k-tiles per pass), psum f32
  - exp((10/256)*sim256) row-sums fused into ScalarE activation accumulator
  - diagonal mask: diag dot recomputed exactly (elementwise prod + ones-mm),
    moved to partition layout via a K=1 outer-product matmul, subtracted
    before ln
  - positives: elementwise prod + ones-mm, reduced on the free axis
"""

import numpy as np

B = 2048
D = 512
N2 = 2 * B              # 4096 total rows
NCORES = 8
RPC = N2 // NCORES      # 512 rows per core
KT = D // 128           # 4 contraction tiles
BLK = 1024              # column-block size for the norm pipeline
NBLK = N2 // BLK        # 4 blocks
TEMP = 0.1
SCALE = 1.0 / TEMP      # 10.0
FP8_SCALE = 16.0        # zn is stored as fp8(zn*16); sim256 = 256*sim
LN_FP8_SCALE = float(np.log(FP8_SCALE))

_CACHE = {}


def _patch_act_tables(nc, mybir):
    """Make Ln and Exp resolve to the shared natural_log_exp_and_others set
    so the compiler emits one ACT table load instead of thrashing."""
    from concourse import hw_specs

    tables = hw_specs.get_activation_tables(nc.m.arch)
    keep = "natural_log_exp_and_others"
    if keep not in tables:
        return
    F = mybir.ActivationFunctionType
    if F.Exp not in tables[keep] or F.Ln not in tables[keep]:
        return
    for name, fns in tables.items():
        if name != keep:
            fns.discard(F.Exp)
            fns.discard(F.Ln)


def _build():
    from concourse import bass, bacc, tile, mybir

    nc = bacc.Bacc("TRN2", target_bir_lowering=False, debug=False,
                   num_devices=NCORES)
    bf16 = mybir.dt.bfloat16
    f32 = mybir.dt.float32
    f8 = mybir.dt.float8e4
    F = mybir.ActivationFunctionType
    A = mybir.AluOpType
    AX = mybir.AxisListType
    DR = mybir.MatmulPerfMode.DoubleRow
    PSUM = bass.MemorySpace.PSUM

    zt = nc.dram_tensor("zt", [D, N2], bf16, kind="ExternalInput").ap()
    zown = nc.dram_tensor("zown", [D, RPC], bf16, kind="ExternalInput").ap()
    zpr = nc.dram_tensor("zpr", [D, RPC], bf16, kind="ExternalInput").ap()
    out = nc.dram_tensor("out", [1, 1], f32, kind="ExternalOutput").ap()

    with tile.TileContext(nc) as tc:
        with (
            tc.tile_pool(name="sb", bufs=1) as sb,
            tc.tile_pool(name="wrk", bufs=2) as wrk,
        ):
            ones = sb.tile([128, 128], bf16, tag="ones")
            nc.vector.memset(ones[:], 1.0)
            bias_ln16 = sb.tile([128, 1], f32, tag="bln16")
            nc.vector.memset(bias_ln16[:], LN_FP8_SCALE)
            bias_10 = sb.tile([128, 1], f32, tag="b10")
            nc.vector.memset(bias_10[:], SCALE)

            # ---- own/partner slices first (main matmul lhsT needs them) ----
            def load_slices(src, tag):
                ts = []
                for k in range(KT):
                    t = sb.tile([128, RPC], bf16, tag=f"{tag}{k}")
                    nc.sync.dma_start(out=t[:], in_=src[k * 128:(k + 1) * 128, :])
                    ts.append(t)
                return ts

            zok = load_slices(zown, "zo")
            zpk = load_slices(zpr, "zp")

            # bf16 residual of the matmul-diagonal around its nominal 256
            diag_row = sb.tile([1, RPC], bf16, tag="diagrow")
            pos_red = sb.tile([128, 1], f32, tag="posr")
            znt = [None] * NBLK                               # fp8(zn*16) blocks

            with (
                tc.tile_pool(name="psA", bufs=2, space=PSUM) as psA,
                tc.tile_pool(name="psA1", bufs=1, space=PSUM) as psA1,
            ):

                def norm_small(tks, tag, fp8_out):
                    lns = wrk.tile([128, RPC], f32, tag="lns_s")
                    rin = wrk.tile([128, RPC], bf16, tag=f"rin_s")
                    ps = psA.tile([128, 512], f32, tag="ssq_s")
                    for k in range(KT):
                        s = wrk.tile([128, RPC], bf16, tag="sq_s")
                        nc.vector.tensor_tensor(s[:], tks[k][:], tks[k][:], A.mult)
                        nc.tensor.matmul(ps[:], ones[:], s[:],
                                         start=(k == 0), stop=(k == KT - 1))
                    nc.scalar.activation(lns[:], ps[:], F.Ln)
                    bias = bias_ln16[:] if fp8_out else 0.0
                    nc.scalar.activation(rin[:], lns[:], F.Exp, scale=-0.5,
                                         bias=bias)
                    if fp8_out:
                        zn = sb.tile([128, KT, RPC], f8, tag=f"zn_{tag}")
                        for k in range(KT):
                            nc.vector.tensor_tensor(zn[:, k, :], tks[k][:],
                                                    rin[:], A.mult)
                    else:
                        zn = sb.tile([128, KT, RPC], bf16, tag=f"zn_{tag}")
                        for k in range(KT):
                            nc.vector.tensor_tensor(zn[:, k, :], tks[k][:],
                                                    rin[:], A.mult)
                    return zn

                zno_t = norm_small(zok, "o", True)
                znp_t = norm_small(zpk, "p", False)   # bf16, x1 scale
                zno = zno_t

                # positives: prod = zno16 * znp -> 16*zn*zn ; colsum-bcast
                pp = psA1.tile([128, 512], f32, tag="pos")
                for k in range(KT):
                    pr = wrk.tile([128, RPC], bf16, tag="prod")
                    nc.vector.tensor_tensor(pr[:], zno[:, k, :], znp_t[:, k, :],
                                            A.mult)
                    nc.tensor.matmul(pp[:], ones[:], pr[:],
                                     start=(k == 0), stop=(k == KT - 1))
                nc.vector.tensor_reduce(pos_red[:], pp[:], AX.X, A.add)

                # diag dots: dg = sum_d (zn16)^2 = 256*|zn|^2, to one row
                dg = psA1.tile([1, 512], f32, tag="diag")
                for k in range(KT):
                    pr = wrk.tile([128, RPC], bf16, tag="prod")
                    nc.vector.tensor_tensor(pr[:], zno[:, k, :], zno[:, k, :],
                                            A.mult)
                    nc.tensor.matmul(dg[:], ones[:, 0:1], pr[:],
                                     start=(k == 0), stop=(k == KT - 1))
                nc.vector.tensor_scalar_add(diag_row[:], dg[:],
                                            -FP8_SCALE ** 2)

                # ---- full z^T per-block pipeline ----
                for b in range(NBLK):
                    bsl = slice(b * BLK, (b + 1) * BLK)
                    zb = []
                    for k in range(KT):
                        t = sb.tile([128, BLK], bf16, tag=f"zt{b}_{k}")
                        nc.sync.dma_start(out=t[:],
                                          in_=zt[k * 128:(k + 1) * 128, bsl])
                        zb.append(t)
                    ps = psA.tile([128, BLK], f32, tag="ssq")
                    for k in range(KT):
                        s = wrk.tile([128, BLK], bf16, tag="sq")
                        nc.vector.tensor_tensor(s[:], zb[k][:], zb[k][:], A.mult)
                        for j in range(BLK // 512):
                            nc.tensor.matmul(ps[:, j * 512:(j + 1) * 512],
                                             ones[:], s[:, j * 512:(j + 1) * 512],
                                             start=(k == 0), stop=(k == KT - 1))
                    lns = wrk.tile([128, BLK], f32, tag="lns")
                    nc.scalar.activation(lns[:], ps[:], F.Ln)
                    rin = wrk.tile([128, BLK], bf16, tag="rin")
                    nc.scalar.activation(rin[:], lns[:], F.Exp, scale=-0.5,
                                         bias=bias_ln16[:])
                    zn = sb.tile([128, KT, BLK], f8, tag=f"znt{b}")
                    for k in range(KT):
                        zn16 = wrk.tile([128, BLK], bf16, tag="zn16")
                        nc.vector.tensor_tensor(zn16[:], zb[k][:], rin[:],
                                                A.mult)
                        nc.gpsimd.tensor_copy(zn[:, k, :], zn16[:])
                    znt[b] = zn

            # ---- main Gram slice (fp8 DoubleRow) + fused exp row-sums ----
            rowp = sb.tile([128, 8], f32, tag="rowp")
            with tc.tile_pool(name="psB", bufs=2, space=PSUM) as psB:
                for h in range(2):          # column half (2048 cols)
                    for m in range(4):      # own-row tile
                        pm = psB.tile([128, 2048], f32, tag="mm")
                        for g in range(KT // 2):    # DoubleRow k-groups
                            lhsT = zno[:, 2 * g:2 * g + 2,
                                       m * 128:(m + 1) * 128]
                            for n4 in range(4):
                                col = h * 2048 + n4 * 512
                                blk, c = col // BLK, col % BLK
                                nc.tensor.matmul(
                                    pm[:, n4 * 512:(n4 + 1) * 512],
                                    lhsT,
                                    znt[blk][:, 2 * g:2 * g + 2, c:c + 512],
                                    start=(g == 0), stop=(g == KT // 2 - 1),
                                    perf_mode=DR)
                        scr = wrk.tile([128, 2048], f32, tag="scr")
                        j = m * 2 + h
                        nc.scalar.activation(scr[:], pm[:], F.Exp,
                                             scale=SCALE / (FP8_SCALE ** 2),
                                             accum_out=rowp[:, j:j + 1])

            # ---- finale ----
            with tc.tile_pool(name="psC", bufs=1, space=PSUM) as psC:
                # move diag row to partition layout via K=1 outer products
                dt = psC.tile([128, 512], f32, tag="dt")
                for m in range(4):
                    nc.tensor.matmul(dt[:, m * 128:(m + 1) * 128],
                                     diag_row[0:1, m * 128:(m + 1) * 128],
                                     ones[0:1, :], start=True, stop=True)
                diag_part = sb.tile([128, 4], f32, tag="diagp")
                for m in range(4):
                    nc.vector.tensor_copy(diag_part[:, m:m + 1],
                                          dt[:, m * 128:m * 128 + 1])
                dexp = sb.tile([128, 4], f32, tag="dexp")
                nc.scalar.activation(dexp[:], diag_part[:], F.Exp,
                                     scale=SCALE / (FP8_SCALE ** 2),
                                     bias=bias_10[:])
                zsum = sb.tile([128, 4], f32, tag="zsum")
                for m in range(4):
                    nc.vector.tensor_tensor(zsum[:, m:m + 1],
                                            rowp[:, 2 * m:2 * m + 1],
                                            rowp[:, 2 * m + 1:2 * m + 2], A.add)
                zarg = sb.tile([128, 4], f32, tag="zarg")
                nc.vector.tensor_tensor(zarg[:], zsum[:], dexp[:], A.subtract)
                logz = sb.tile([128, 5], f32, tag="logz")
                nc.scalar.activation(logz[:, 0:4], zarg[:], F.Ln)
                # pos term: pos_red = 16*sum_r cos_r on every partition
                nc.vector.tensor_scalar_mul(
                    logz[:, 4:5], pos_red[:], -SCALE / FP8_SCALE / 128.0)
                red1 = sb.tile([128, 1], f32, tag="red1")
                nc.vector.tensor_reduce(red1[:], logz[:], AX.X, A.add)
                fin = sb.tile([1, 1], f32, tag="fin")
                nc.gpsimd.tensor_reduce(fin[:], red1[:], AX.C, A.add)
                nc.sync.dma_start(out=out, in_=fin[:])

    _patch_act_tables(nc, mybir)
    nc.compile()
    return nc


def _get_nc():
    if "nc" not in _CACHE:
        _CACHE["nc"] = _build()
    return _CACHE["nc"]


def _in_maps(z_i, z_j):
    import ml_dtypes

    z = np.concatenate(
        [np.asarray(z_i, np.float32), np.asarray(z_j, np.float32)], axis=0)
    zt = np.ascontiguousarray(z.T).astype(ml_dtypes.bfloat16)
    maps = []
    for c in range(NCORES):
        o = c * RPC
        po = (o + B) % N2
        maps.append({
            "zt": zt,
            "zown": np.ascontiguousarray(zt[:, o:o + RPC]),
            "zpr": np.ascontiguousarray(zt[:, po:po + RPC]),
        })
    return maps


def _run(z_i, z_j, trace=False):
    from concourse.bass_utils import run_bass_kernel_spmd

    nc = _get_nc()
    return run_bass_kernel_spmd(nc, _in_maps(z_i, z_j), list(range(NCORES)),
                                trace=trace)


def kernel(z_i, z_j):
    res = _run(z_i, z_j, trace=False)
    total = sum(float(r["out"][0, 0]) for r in res.results)
    return np.float32(total / N2)


# revision 11
# speedup vs baseline: 1.5185x; 1.5185x over previous
"""NT-Xent (SimCLR contrastive) loss on Trainium2, sharded across 8 NeuronCores.

Sharding: each core computes a [512, 4096] row-slice of the similarity matrix.
Host ships z^T (bf16, transposed layout only — no host arithmetic) shared to
all cores plus per-core own/partner column slices; per-core scalar partials
are summed on the host (the unshard step).

Device pipeline (per core, SPMD):
  - column norms: squares (GpSimd) + all-ones matmul = partition-reduce with
    free broadcast; rinv16 = exp(-0.5*ln(ssq) + ln 16)  [one ACT table set]
  - zn16 = z * rinv16 -> fp8e4 (x16 scaling keeps values out of fp8 denormals)
  - main Gram slice: fp8 DoubleRow matmuls (2 k-tiles per pass), psum f32
  - exp((10/256)*sim256) row-sums fused into ScalarE activation accumulator
  - diagonal mask: diag dot recomputed exactly (elementwise prod + ones-mm),
    moved to partition layout via a K=1 outer-product matmul, subtracted
    before ln
  - positives: elementwise prod + ones-mm, reduced on the free axis
"""

import numpy as np

B = 2048
D = 512
N2 = 2 * B              # 4096 total rows
NCORES = 8
RPC = N2 // NCORES      # 512 rows per core
KT = D // 128           # 4 contraction tiles
BLK = 1024              # column-block size for the norm pipeline
NBLK = N2 // BLK        # 4 blocks
TEMP = 0.1
SCALE = 1.0 / TEMP      # 10.0
FP8_SCALE = 16.0        # zn is stored as fp8(zn*16); sim256 = 256*sim
LN_FP8_SCALE = float(np.log(FP8_SCALE))

_CACHE = {}


def _patch_act_tables(nc, mybir):
    """Make Ln and Exp resolve to the shared natural_log_exp_and_others set
    so the compiler emits one ACT table load instead of thrashing."""
    from concourse import hw_specs

    tables = hw_specs.get_activation_tables(nc.m.arch)
    keep = "natural_log_exp_and_others"
    if keep not in tables:
        return
    F = mybir.ActivationFunctionType
    if F.Exp not in tables[keep] or F.Ln not in tables[keep]:
        return
    for name, fns in tables.items():
        if name != keep:
            fns.discard(F.Exp)
            fns.discard(F.Ln)


def _build():
    from concourse import bass, bacc, tile, mybir

    nc = bacc.Bacc("TRN2", target_bir_lowering=False, debug=False,
                   num_devices=NCORES)
    bf16 = mybir.dt.bfloat16
    f32 = mybir.dt.float32
    f8 = mybir.dt.float8e4
    F = mybir.ActivationFunctionType
    A = mybir.AluOpType
    AX = mybir.AxisListType
    DR = mybir.MatmulPerfMode.DoubleRow
    PSUM = bass.MemorySpace.PSUM

    zt = nc.dram_tensor("zt", [D, N2], bf16, kind="ExternalInput").ap()
    zown = nc.dram_tensor("zown", [D, RPC], bf16, kind="ExternalInput").ap()
    zpr = nc.dram_tensor("zpr", [D, RPC], bf16, kind="ExternalInput").ap()
    out = nc.dram_tensor("out", [1, 1], f32, kind="ExternalOutput").ap()

    with tile.TileContext(nc) as tc:
        with (
            tc.tile_pool(name="sb", bufs=1) as sb,
            tc.tile_pool(name="wrk", bufs=2) as wrk,
        ):
            ones = sb.tile([128, 128], bf16, tag="ones")
            nc.vector.memset(ones[:], 1.0)
            bias_ln16 = sb.tile([128, 1], f32, tag="bln16")
            nc.vector.memset(bias_ln16[:], LN_FP8_SCALE)
            bias_10 = sb.tile([128, 1], f32, tag="b10")
            nc.vector.memset(bias_10[:], SCALE)

            # ---- own/partner slices first (main matmul lhsT needs them) ----
            def load_slices(src, tag):
                ts = []
                for k in range(KT):
                    t = sb.tile([128, RPC], bf16, tag=f"{tag}{k}")
                    nc.sync.dma_start(out=t[:], in_=src[k * 128:(k + 1) * 128, :])
                    ts.append(t)
                return ts

            zok = load_slices(zown, "zo")
            zpk = load_slices(zpr, "zp")

            # bf16 residual of the matmul-diagonal around its nominal 256
            diag_row = sb.tile([1, RPC], bf16, tag="diagrow")
            pos_red = sb.tile([128, 1], f32, tag="posr")
            znt = [None] * NBLK                               # fp8(zn*16) blocks

            with (
                tc.tile_pool(name="psA", bufs=2, space=PSUM) as psA,
                tc.tile_pool(name="psA1", bufs=1, space=PSUM) as psA1,
            ):

                def norm_small(tks, tag, fp8_out):
                    lns = wrk.tile([128, RPC], f32, tag="lns_s")
                    rin = wrk.tile([128, RPC], bf16, tag=f"rin_s")
                    ps = psA.tile([128, 512], f32, tag="ssq_s")
                    for k in range(KT):
                        s = wrk.tile([128, RPC], bf16, tag="sq_s")
                        nc.vector.tensor_tensor(s[:], tks[k][:], tks[k][:], A.mult)
                        nc.tensor.matmul(ps[:], ones[:], s[:],
                                         start=(k == 0), stop=(k == KT - 1))
                    nc.scalar.activation(lns[:], ps[:], F.Ln)
                    bias = bias_ln16[:] if fp8_out else 0.0
                    nc.scalar.activation(rin[:], lns[:], F.Exp, scale=-0.5,
                                         bias=bias)
                    if fp8_out:
                        zn = sb.tile([128, KT, RPC], f8, tag=f"zn_{tag}")
                        for k in range(KT):
                            nc.vector.tensor_tensor(zn[:, k, :], tks[k][:],
                                                    rin[:], A.mult)
                    else:
                        zn = sb.tile([128, KT, RPC], bf16, tag=f"zn_{tag}")
                        for k in range(KT):
                            nc.vector.tensor_tensor(zn[:, k, :], tks[k][:],
                                                    rin[:], A.mult)
                    return zn

                zno_t = norm_small(zok, "o", True)
                znp_t = norm_small(zpk, "p", False)   # bf16, x1 scale
                zno = zno_t

                # positives: prod = zno16 * znp -> 16*zn*zn ; colsum-bcast
                pp = psA1.tile([128, 512], f32, tag="pos")
                for k in range(KT):
                    pr = wrk.tile([128, RPC], bf16, tag="prod")
                    nc.vector.tensor_tensor(pr[:], zno[:, k, :], znp_t[:, k, :],
                                            A.mult)
                    nc.tensor.matmul(pp[:], ones[:], pr[:],
                                     start=(k == 0), stop=(k == KT - 1))
                nc.vector.tensor_reduce(pos_red[:], pp[:], AX.X, A.add)

                # diag dots: dg = sum_d (zn16)^2 = 256*|zn|^2, to one row
                dg = psA1.tile([1, 512], f32, tag="diag")
                for k in range(KT):
                    pr = wrk.tile([128, RPC], bf16, tag="prod")
                    nc.vector.tensor_tensor(pr[:], zno[:, k, :], zno[:, k, :],
                                            A.mult)
                    nc.tensor.matmul(dg[:], ones[:, 0:1], pr[:],
                                     start=(k == 0), stop=(k == KT - 1))
                nc.vector.tensor_scalar_add(diag_row[:], dg[:],
                                            -FP8_SCALE ** 2)

                # ---- full z^T per-block pipeline ----
                for b in range(NBLK):
                    bsl = slice(b * BLK, (b + 1) * BLK)
                    zb = []
                    for k in range(KT):
                        t = sb.tile([128, BLK], bf16, tag=f"zt{b}_{k}")
                        nc.sync.dma_start(out=t[:],
                                          in_=zt[k * 128:(k + 1) * 128, bsl])
                        zb.append(t)
                    ps = psA.tile([128, BLK], f32, tag="ssq")
                    for k in range(KT):
                        s = wrk.tile([128, BLK], bf16, tag="sq")
                        nc.vector.tensor_tensor(s[:], zb[k][:], zb[k][:], A.mult)
                        for j in range(BLK // 512):
                            nc.tensor.matmul(ps[:, j * 512:(j + 1) * 512],
                                             ones[:], s[:, j * 512:(j + 1) * 512],
                                             start=(k == 0), stop=(k == KT - 1))
                    lns = wrk.tile([128, BLK], f32, tag="lns")
                    nc.scalar.activation(lns[:], ps[:], F.Ln)
                    rin = wrk.tile([128, BLK], bf16, tag="rin")
                    nc.scalar.activation(rin[:], lns[:], F.Exp, scale=-0.5,
                                         bias=bias_ln16[:])
                    zn16 = wrk.tile([128, KT, BLK], bf16, tag="zn16")
                    for k in range(KT):
                        nc.vector.tensor_tensor(zn16[:, k, :], zb[k][:], rin[:],
                                                A.mult)
                    zn = sb.tile([128, KT, BLK], f8, tag=f"znt{b}")
                    nc.gpsimd.dma_start(out=zn[:], in_=zn16[:])
                    znt[b] = zn

            # ---- main Gram slice (fp8 DoubleRow) + fused exp row-sums ----
            rowp = sb.tile([128, 8], f32, tag="rowp")
            with tc.tile_pool(name="psB", bufs=2, space=PSUM) as psB:
                for h in range(2):          # column half (2048 cols)
                    for m in range(4):      # own-row tile
                        pm = psB.tile([128, 2048], f32, tag="mm")
                        for g in range(KT // 2):    # DoubleRow k-groups
                            lhsT = zno[:, 2 * g:2 * g + 2,
                                       m * 128:(m + 1) * 128]
                            for n4 in range(4):
                                col = h * 2048 + n4 * 512
                                blk, c = col // BLK, col % BLK
                                nc.tensor.matmul(
                                    pm[:, n4 * 512:(n4 + 1) * 512],
                                    lhsT,
                                    znt[blk][:, 2 * g:2 * g + 2, c:c + 512],
                                    start=(g == 0), stop=(g == KT // 2 - 1),
                                    perf_mode=DR)
                        scr = wrk.tile([128, 2048], f32, tag="scr")
                        j = m * 2 + h
                        nc.scalar.activation(scr[:], pm[:], F.Exp,
                                             scale=SCALE / (FP8_SCALE ** 2),
                                             accum_out=rowp[:, j:j + 1])

            # ---- finale ----
            with tc.tile_pool(name="psC", bufs=1, space=PSUM) as psC:
                # move diag row to partition layout via K=1 outer products
                dt = psC.tile([128, 512], f32, tag="dt")
                for m in range(4):
                    nc.tensor.matmul(dt[:, m * 128:(m + 1) * 128],
                                     diag_row[0:1, m * 128:(m + 1) * 128],
                                     ones[0:1, :], start=True, stop=True)
                diag_part = sb.tile([128, 4], f32, tag="diagp")
                for m in range(4):
                    nc.vector.tensor_copy(diag_part[:, m:m + 1],
                                          dt[:, m * 128:m * 128 + 1])
                dexp = sb.tile([128, 4], f32, tag="dexp")
                nc.scalar.activation(dexp[:], diag_part[:], F.Exp,
                                     scale=SCALE / (FP8_SCALE ** 2),
                                     bias=bias_10[:])
                zsum = sb.tile([128, 4], f32, tag="zsum")
                for m in range(4):
                    nc.vector.tensor_tensor(zsum[:, m:m + 1],
                                            rowp[:, 2 * m:2 * m + 1],
                                            rowp[:, 2 * m + 1:2 * m + 2], A.add)
                zarg = sb.tile([128, 4], f32, tag="zarg")
                nc.vector.tensor_tensor(zarg[:], zsum[:], dexp[:], A.subtract)
                logz = sb.tile([128, 5], f32, tag="logz")
                nc.scalar.activation(logz[:, 0:4], zarg[:], F.Ln)
                # pos term: pos_red = 16*sum_r cos_r on every partition
                nc.vector.tensor_scalar_mul(
                    logz[:, 4:5], pos_red[:], -SCALE / FP8_SCALE / 128.0)
                red1 = sb.tile([128, 1], f32, tag="red1")
                nc.vector.tensor_reduce(red1[:], logz[:], AX.X, A.add)
                fin = sb.tile([1, 1], f32, tag="fin")
                nc.gpsimd.tensor_reduce(fin[:], red1[:], AX.C, A.add)
                nc.sync.dma_start(out=out, in_=fin[:])

    _patch_act_tables(nc, mybir)
    nc.compile()
    return nc


def _get_nc():
    if "nc" not in _CACHE:
        _CACHE["nc"] = _build()
    return _CACHE["nc"]


def _in_maps(z_i, z_j):
    import ml_dtypes

    z = np.concatenate(
        [np.asarray(z_i, np.float32), np.asarray(z_j, np.float32)], axis=0)
    zt = np.ascontiguousarray(z.T).astype(ml_dtypes.bfloat16)
    maps = []
    for c in range(NCORES):
        o = c * RPC
        po = (o + B) % N2
        maps.append({
            "zt": zt,
            "zown": np.ascontiguousarray(zt[:, o:o + RPC]),
            "zpr": np.ascontiguousarray(zt[:, po:po + RPC]),
        })
    return maps


def _run(z_i, z_j, trace=False):
    from concourse.bass_utils import run_bass_kernel_spmd

    nc = _get_nc()
    return run_bass_kernel_spmd(nc, _in_maps(z_i, z_j), list(range(NCORES)),
                                trace=trace)


def kernel(z_i, z_j):
    res = _run(z_i, z_j, trace=False)
    total = sum(float(r["out"][0, 0]) for r in res.results)
    return np.float32(total / N2)
